# revision 2
# baseline (speedup 1.0000x reference)
"""Trainium2 Bass kernel for the DEN (Mahalanobis distance) layer.

Computes out[b, e] = (x_b - c_e)^T Sigma_e^{-1} (x_b - c_e) for
x [8192, 128], Centroids [128, 1, 128], Sigmas [128, 128, 128].

Strategy (v2)
-------------
Expand the quadratic form with A_e = sym(Sigma_e^{-1}):

    out[b, e] = x_b^T A_e x_b - 2 (A_e c_e) . x_b + c_e^T A_e c_e

and decompose x^T A x over the wrapped diagonals of A (j = 0..64, each
unordered pair covered once; factor 2 for 1<=j<=63).  The shifted
products P_j[d, b] = xT[d, b] * xT[(d+j)%128, b] feed a chain of
PSUM-accumulated [128,128]x[128,512] matmuls with host-precomputed
coefficient packs.

v2 changes vs v1 (59.7us -> target ~40us):
  *  Rotated copies of xT are built ON-CHIP via SBUF->SBUF DMA
     (2 transfers per rotation: main + wrap) instead of shipping
     3.84 MB of host-rotated data through HBM.  HBM input drops from
     6.4 MB to ~1.4 MB per core.
  *  Off-diagonal coefficient packs (j=1..64) are fp8e4 (stationary
     operand only; the moving product operand stays bf16, so the PE
     still runs 1 cycle/row).  Halves the coefficient DMA bytes.
     Pack 0 (the large diagonal coefficients) and the linear term stay
     bf16 for accuracy.
  *  DMA triggers are split across the two HWDGE queues (Sync +
     Scalar) so trigger issue is not serialized on one engine.
  *  The 8 tail product packs (consumed last by the PE) are computed
     on the GpSimd/Pool engine, shaving the DVE product stream.

Sharding: data-parallel over batch B across the 8 cores (1024 rows
each); coefficient packs (derived from Sigmas/Centroids) replicated.
"""

import os
import sys

sys.path.insert(0, "/opt/trn_rl_repo")

import numpy as np
import ml_dtypes

E, B, D = 128, 8192, 128
NCORES = 8
BLOC = B // NCORES          # 1024 batch rows per core
BT = 512                    # matmul free-dim tile (one PSUM bank)
NPACK = 65                  # wrapped diagonals j = 0..64
NSLOT = 16                  # rotation slots: 0..7 then 8,16,...,64
ROTVALS = (1, 2, 3, 4, 5, 6, 7) + tuple(range(8, 65, 8))
CHUNK = 16                  # fp8 coefficient packs per DMA chunk
NCHUNK = 4                  # 64 / 16

# mixed-radix product groups: (in0 slot range [lo,hi), in1 slot).
# Column i of a group is the product rot[lo+i] * rot[in1], covering
# diagonal j = rotval(in1) - rotval(lo+i) with row rotation a = lo+i.
# The first 10 groups run on the vector engine (56 packs); the last 4
# (8 packs, consumed last by the PE chain) run on GpSimd.
GROUPS = [(0, 1, 1), (0, 1, 2), (0, 2, 4), (0, 4, 8)] \
         + [(0, 8, 8 + k) for k in range(1, 7)] \
         + [(2, 5, 15), (5, 8, 15), (1, 2, 15), (0, 1, 15)]
NGROUP_DVE = 10             # groups [0..9] on DVE, [10..13] on Pool


def _slotval(s):
    return s if s <= 7 else 8 * (s - 7)


# pack emission order: j=0 (scalar-engine square), then the group columns;
# AVAL[pos] is the row rotation of the product feeding that pack
ORDER = [0]
AVAL = [0]
for _lo, _hi, _s1 in GROUPS:
    for _i in range(_hi - _lo):
        ORDER.append(_slotval(_s1) - (_lo + _i))
        AVAL.append(_lo + _i)

bf16 = ml_dtypes.bfloat16
f8e4 = ml_dtypes.float8_e4m3

_STATE: dict = {}


def _build_module():
    import concourse.bacc as bacc
    import concourse.tile as tile
    import concourse.mybir as mybir
    from contextlib import ExitStack

    nc = bacc.Bacc("TRN2", target_bir_lowering=False, debug=False)

    xT_d = nc.dram_tensor("xT", [D, BLOC], mybir.dt.bfloat16, kind="ExternalInput")
    cw0_d = nc.dram_tensor("cw0", [D, E], mybir.dt.bfloat16, kind="ExternalInput")
    cwf_d = nc.dram_tensor("cwf", [D, (NPACK - 1) * E], mybir.dt.float8e4,
                           kind="ExternalInput")
    um_d = nc.dram_tensor("um", [D, E], mybir.dt.bfloat16, kind="ExternalInput")
    tv_d = nc.dram_tensor("tv", [E, 1], mybir.dt.float32, kind="ExternalInput")
    out_d = nc.dram_tensor("out", [E, BLOC], mybir.dt.float32, kind="ExternalOutput")

    f32 = mybir.dt.float32
    b16 = mybir.dt.bfloat16
    Ident = mybir.ActivationFunctionType.Identity

    with tile.TileContext(nc) as tc, ExitStack() as ctx:
        const_pool = ctx.enter_context(tc.tile_pool(name="const", bufs=1))
        coef_pool = ctx.enter_context(tc.tile_pool(name="coef", bufs=NCHUNK))
        p0_pool = ctx.enter_context(tc.tile_pool(name="p0", bufs=1))
        g_pool = ctx.enter_context(tc.tile_pool(name="g", bufs=4))
        gp_pool = ctx.enter_context(tc.tile_pool(name="gp", bufs=4))
        psum_pool = ctx.enter_context(tc.tile_pool(name="acc", bufs=1, space="PSUM"))
        out_pool = ctx.enter_context(tc.tile_pool(name="outs", bufs=2))

        ROTS = const_pool.tile([D, NSLOT * BLOC], b16, tag="rots")
        R3 = ROTS[:, :].rearrange("p (s b) -> p s b", s=NSLOT)
        UM = const_pool.tile([D, E], b16, tag="um")
        CW0 = const_pool.tile([D, E], b16, tag="cw0")
        TV = const_pool.tile([E, 1], f32, tag="tv")
        coef_tiles = [coef_pool.tile([D, CHUNK * E], mybir.dt.float8e4,
                                     name=f"cwf{ci}", tag="cwf")
                      for ci in range(NCHUNK)]

        # --- DMA plan ---------------------------------------------------
        # Sync queue: xT, rotation mains (consumption order, slot 15 early
        # for the Pool groups), fp8 coefficient chunks interleaved, tv.
        # Scalar queue: um, cw0, rotation wraps; square + eviction ride on
        # the same queue.
        nc.sync.dma_start(ROTS[:, 0:BLOC], xT_d.ap())
        nc.scalar.dma_start(UM[:], um_d.ap())
        nc.scalar.dma_start(CW0[:], cw0_d.ap())

        def rot_main(s):
            r = ROTVALS[s - 1]
            nc.sync.dma_start(ROTS[0:D - r, s * BLOC:(s + 1) * BLOC],
                              ROTS[r:D, 0:BLOC])

        def rot_wrap(s):
            r = ROTVALS[s - 1]
            nc.scalar.dma_start(ROTS[D - r:D, s * BLOC:(s + 1) * BLOC],
                                ROTS[0:r, 0:BLOC])

        def dma_coef(ci):
            nc.sync.dma_start(coef_tiles[ci][:],
                              cwf_d.ap()[:, ci * CHUNK * E:(ci + 1) * CHUNK * E])

        # rotation slot issue order (consumption order; 15 early for Pool)
        ROT_ORDER = [1, 2, 15, 4, 3, 8, 5, 6, 7, 9, 10, 11, 12, 13, 14]
        sync_plan = ROT_ORDER[:3] + ["c0"] + ROT_ORDER[3:5] + ["c1"] \
            + ROT_ORDER[5:9] + ["c2"] + ROT_ORDER[9:11] + ["c3"] \
            + ROT_ORDER[11:]
        for item in sync_plan:
            if isinstance(item, str):
                dma_coef(int(item[1:]))
            else:
                rot_main(item)
        nc.sync.dma_start(TV[:], tv_d.ap())

        # pack 0 products: x^2 on the scalar engine (before the wraps so
        # the PE's pos-0 matmul isn't stuck behind 15 wrap triggers)
        PK0 = p0_pool.tile([D, BLOC], b16)
        nc.scalar.square(PK0[:, :], ROTS[:, 0:BLOC])
        for s in ROT_ORDER:
            rot_wrap(s)

        nbt = BLOC // BT
        psums = []
        for bt in range(nbt):
            ps = psum_pool.tile([E, BT], f32, tag=f"ps{bt}", name=f"ps{bt}")
            psums.append(ps)

        # linear term first so the accumulation chain can start immediately
        for bt in range(nbt):
            nc.tensor.matmul(psums[bt][:, :], UM[:, :],
                             ROTS[:, bt * BT:bt * BT + BT],
                             start=True, stop=False)

        def emit_matmuls(pos, rhs_tile, col_base):
            if pos == 0:
                lhsT = CW0[:, :]
            else:
                ci, cc = divmod(pos - 1, CHUNK)
                lhsT = coef_tiles[ci][:, cc * E:(cc + 1) * E]
            for bt in range(nbt):
                nc.tensor.matmul(
                    psums[bt][:, :],
                    lhsT,
                    rhs_tile[:, col_base + bt * BT:col_base + bt * BT + BT],
                    start=False,
                    stop=(pos == NPACK - 1),
                )

        emit_matmuls(0, PK0, 0)

        # product groups: Pool (gpsimd) groups emitted first so the slow
        # engine starts as soon as its rotations land; PE consumes them
        # last.  DVE groups follow in PE consumption order.
        pool_tiles = {}
        for gi in range(NGROUP_DVE, len(GROUPS)):
            lo, hi, s1 = GROUPS[gi]
            w = hi - lo
            PKg = gp_pool.tile([D, w * BLOC], b16, name=f"PKp{gi}", tag="gp")
            nc.gpsimd.tensor_mul(
                PKg[:, :].rearrange("p (s b) -> p s b", s=w),
                R3[:, lo:hi, :],
                R3[:, s1:s1 + 1, :].broadcast_to((D, w, BLOC)),
            )
            pool_tiles[gi] = PKg

        pos = 1
        for gi, (lo, hi, s1) in enumerate(GROUPS):
            w = hi - lo
            if gi < NGROUP_DVE:
                PKg = g_pool.tile([D, w * BLOC], b16, name=f"PKg{gi}", tag="g")
                nc.vector.tensor_mul(
                    PKg[:, :].rearrange("p (s b) -> p s b", s=w),
                    R3[:, lo:hi, :],
                    R3[:, s1:s1 + 1, :].broadcast_to((D, w, BLOC)),
                )
            else:
                PKg = pool_tiles[gi]
            for i in range(w):
                emit_matmuls(pos, PKg, i * BLOC)
                pos += 1

        # evict both PSUM chains into one SBUF tile (vector + scalar engine
        # in parallel), then a single output DMA trigger
        OT = out_pool.tile([E, BLOC], f32)
        nc.vector.tensor_scalar_add(OT[:, 0:BT], psums[0][:, :], TV[:, 0:1])
        nc.scalar.activation(OT[:, BT:BLOC], psums[1][:, :], Ident,
                             bias=TV[:, 0:1])
        nc.sync.dma_start(out_d.ap()[:, :], OT[:, :])

    nc.compile()
    return nc


def _host_precompute(Centroids: np.ndarray, Sigmas: np.ndarray):
    """Coefficient packs from the (replicated) small parameters."""
    Sinv = np.linalg.inv(Sigmas.astype(np.float64))
    A = 0.5 * (Sinv + np.swapaxes(Sinv, 1, 2))          # [E, D, D] symmetric
    c = Centroids[:, 0, :].astype(np.float64)           # [E, D]
    Ac = np.einsum("edk,ek->ed", A, c)

    cw = np.zeros((D, NPACK, E), np.float32)            # [row, emission pos, e]
    idx = np.arange(D)
    for posi, j in enumerate(ORDER):
        s = 2.0 if 1 <= j <= 63 else 1.0
        a = AVAL[posi]
        cw[:, posi, :] = s * A[:, (idx + a) % D, (idx + a + j) % D].T
    cw0_host = np.ascontiguousarray(cw[:, 0, :]).astype(bf16)            # [D, E]
    cwf_host = np.ascontiguousarray(
        cw[:, 1:, :].reshape(D, (NPACK - 1) * E)
    ).astype(f8e4)                                                       # [D, 64*E]
    um_host = np.ascontiguousarray((-2.0 * Ac.T)).astype(bf16)           # [D, E]
    tv_host = np.ascontiguousarray(
        np.einsum("ed,ed->e", Ac, c).astype(np.float32)[:, None]
    )                                                                    # [E, 1]
    return cw0_host, cwf_host, um_host, tv_host


def _get_nc():
    if "nc" not in _STATE:
        os.environ.setdefault("JAX_COMPILATION_CACHE_DIR", "/root/.jax_cache")
        _STATE["nc"] = _build_module()
    return _STATE["nc"]


def _make_in_maps(x, Centroids, Sigmas):
    cw0_host, cwf_host, um_host, tv_host = _host_precompute(
        np.asarray(Centroids, np.float32), np.asarray(Sigmas, np.float32)
    )
    xT = np.ascontiguousarray(np.asarray(x, np.float32).T).astype(bf16)  # [D, B]
    in_maps = []
    for cidx in range(NCORES):
        xTs = np.ascontiguousarray(xT[:, cidx * BLOC:(cidx + 1) * BLOC])
        in_maps.append({
            "xT": xTs,
            "cw0": cw0_host,
            "cwf": cwf_host,
            "um": um_host,
            "tv": tv_host,
        })
    return in_maps


def _run_device(in_maps, trace=False):
    from concourse import bass_utils

    nc = _get_nc()
    return bass_utils.run_bass_kernel_spmd(
        nc, in_maps, core_ids=list(range(NCORES)), trace=trace
    )


def kernel(x, Centroids, Sigmas):
    in_maps = _make_in_maps(x, Centroids, Sigmas)
    res = _run_device(in_maps)
    outT = np.concatenate([res.results[c]["out"] for c in range(NCORES)], axis=1)
    return np.ascontiguousarray(outT.T).astype(np.float32)


# revision 3
# speedup vs baseline: 1.4717x; 1.4717x over previous
"""Trainium2 Bass kernel for the DEN (Mahalanobis distance) layer.

Computes out[b, e] = (x_b - c_e)^T Sigma_e^{-1} (x_b - c_e) for
x [8192, 128], Centroids [128, 1, 128], Sigmas [128, 128, 128].

Strategy
--------
Expand the quadratic form with A_e = sym(Sigma_e^{-1}):

    out[b, e] = x_b^T A_e x_b - 2 (A_e c_e) . x_b + c_e^T A_e c_e

and decompose x^T A x over the wrapped diagonals of A (j = 0..64; each
unordered pair lands in exactly one diagonal, factor 2 for 1<=j<=63).
The shifted products P_j[d, b] = xT[d, b] * xT[(d+j)%128, b] feed a
chain of PSUM-accumulated [128,128]x[128,512] matmuls with
host-precomputed coefficient packs; the constant term rides in as the
activation bias during PSUM->SBUF eviction.

Shifts come from host-prebuilt partition-rotated copies of xT (a
difference set of rotations {0..7, 8, 16, ..., 64} covers every
j = b - a in 0..64); the row rotation is absorbed into the coefficient
packs on the host.  All rotations live in one SBUF mega-tile, so
diagonals sharing an operand merge into a single strided vector-engine
op: 14 tensor_mul ops + 1 scalar-engine square.

v3 tuning (from v1's 59.7us trace):
  *  Off-diagonal coefficient packs (j=1..64) are fp8e4 — stationary
     operand only, moving stays bf16, PE speed unchanged.  Halves the
     coefficient HBM bytes (2.13 MB -> 1.06 MB + 64 KB bf16).
  *  Coefficient/constant DMAs ride the Scalar HWDGE queue; the Sync
     queue carries only xT + the 15 rotations + the output.  In v1 all
     shared one queue and the first coefficient chunk landed ~7us late,
     stalling the PE 9us.
  *  Products all on DVE (GpSimd tensor ops measured ~6us launch
     overhead each — not viable), rotations shipped from the host
     (SBUF->SBUF rotation DMAs measured as stealing DVE SBUF ports).

Sharding: data-parallel over batch B across the 8 cores (1024 rows
each); coefficient packs (derived from Sigmas/Centroids) replicated.
"""

import os
import sys

sys.path.insert(0, "/opt/trn_rl_repo")

import numpy as np
import ml_dtypes

E, B, D = 128, 8192, 128
NCORES = 8
BLOC = B // NCORES          # 1024 batch rows per core
BT = 512                    # matmul free-dim tile (one PSUM bank)
NPACK = 65                  # wrapped diagonals j = 0..64
NSLOT = 16                  # rotation slots: 0..7 then 8,16,...,64
ROTVALS = (1, 2, 3, 4, 5, 6, 7) + tuple(range(8, 65, 8))
CHUNK = 16                  # fp8 coefficient packs per DMA chunk
NCHUNK = 4                  # 64 / 16

# mixed-radix product groups: (in0 slot range [lo,hi), in1 slot).
# Column i of a group is the product rot[lo+i] * rot[in1], covering
# diagonal j = rotval(in1) - rotval(lo+i) with row rotation a = lo+i.
GROUPS = [(0, 1, 1), (0, 1, 2), (0, 2, 4), (0, 4, 8)] \
         + [(0, 8, 8 + k) for k in range(1, 7)] \
         + [(2, 5, 15), (5, 8, 15), (1, 2, 15), (0, 1, 15)]


def _slotval(s):
    return s if s <= 7 else 8 * (s - 7)


# pack emission order: j=0 (scalar-engine square), then the group columns;
# AVAL[pos] is the row rotation of the product feeding that pack
ORDER = [0]
AVAL = [0]
for _lo, _hi, _s1 in GROUPS:
    for _i in range(_hi - _lo):
        ORDER.append(_slotval(_s1) - (_lo + _i))
        AVAL.append(_lo + _i)

bf16 = ml_dtypes.bfloat16
f8e4 = ml_dtypes.float8_e4m3

_STATE: dict = {}


def _build_module():
    import concourse.bacc as bacc
    import concourse.tile as tile
    import concourse.mybir as mybir
    from contextlib import ExitStack

    nc = bacc.Bacc("TRN2", target_bir_lowering=False, debug=False)

    xT_d = nc.dram_tensor("xT", [D, BLOC], mybir.dt.bfloat16, kind="ExternalInput")
    xr_d = nc.dram_tensor("xrot", [NSLOT - 1, D, BLOC], mybir.dt.bfloat16,
                          kind="ExternalInput")
    cw0_d = nc.dram_tensor("cw0", [D, E], mybir.dt.bfloat16, kind="ExternalInput")
    cwf_d = nc.dram_tensor("cwf", [D, (NPACK - 1) * E], mybir.dt.float8e4,
                           kind="ExternalInput")
    um_d = nc.dram_tensor("um", [D, E], mybir.dt.bfloat16, kind="ExternalInput")
    tv_d = nc.dram_tensor("tv", [E, 1], mybir.dt.float32, kind="ExternalInput")
    out_d = nc.dram_tensor("out", [E, BLOC], mybir.dt.float32, kind="ExternalOutput")

    f32 = mybir.dt.float32
    b16 = mybir.dt.bfloat16
    Ident = mybir.ActivationFunctionType.Identity

    with tile.TileContext(nc) as tc, ExitStack() as ctx:
        const_pool = ctx.enter_context(tc.tile_pool(name="const", bufs=1))
        coef_pool = ctx.enter_context(tc.tile_pool(name="coef", bufs=NCHUNK))
        p0_pool = ctx.enter_context(tc.tile_pool(name="p0", bufs=1))
        g_pool = ctx.enter_context(tc.tile_pool(name="g", bufs=5))
        psum_pool = ctx.enter_context(tc.tile_pool(name="acc", bufs=1, space="PSUM"))
        out_pool = ctx.enter_context(tc.tile_pool(name="outs", bufs=2))

        ROTS = const_pool.tile([D, NSLOT * BLOC], b16, tag="rots")
        R3 = ROTS[:, :].rearrange("p (s b) -> p s b", s=NSLOT)
        UM = const_pool.tile([D, E], b16, tag="um")
        CW0 = const_pool.tile([D, E], b16, tag="cw0")
        TV = const_pool.tile([E, 1], f32, tag="tv")
        coef_tiles = [coef_pool.tile([D, CHUNK * E], mybir.dt.float8e4,
                                     name=f"cwf{ci}", tag="cwf")
                      for ci in range(NCHUNK)]

        # --- DMA plan ---------------------------------------------------
        # Scalar HWDGE queue: all coefficient/constant traffic (no waits,
        # lands in the first few us), then the pack-0 square + one
        # eviction half.  Sync queue: xT + rotations in consumption
        # order + the output store.
        nc.scalar.dma_start(UM[:], um_d.ap())
        nc.scalar.dma_start(CW0[:], cw0_d.ap())
        for ci in range(NCHUNK):
            nc.scalar.dma_start(
                coef_tiles[ci][:],
                cwf_d.ap()[:, ci * CHUNK * E:(ci + 1) * CHUNK * E])
        nc.scalar.dma_start(TV[:], tv_d.ap())

        nc.sync.dma_start(ROTS[:, 0:BLOC], xT_d.ap())

        def dma_rot(s):
            nc.sync.dma_start(ROTS[:, s * BLOC:(s + 1) * BLOC], xr_d.ap()[s - 1])

        for s in [1, 2, 4, 3, 8, 5, 6, 7, 9, 10, 11, 12, 13, 14, 15]:
            dma_rot(s)

        nbt = BLOC // BT
        psums = []
        for bt in range(nbt):
            ps = psum_pool.tile([E, BT], f32, tag=f"ps{bt}", name=f"ps{bt}")
            psums.append(ps)

        # linear term first so the accumulation chain can start immediately
        for bt in range(nbt):
            nc.tensor.matmul(psums[bt][:, :], UM[:, :],
                             ROTS[:, bt * BT:bt * BT + BT],
                             start=True, stop=False)

        def emit_matmuls(pos, rhs_tile, col_base):
            if pos == 0:
                lhsT = CW0[:, :]
            else:
                ci, cc = divmod(pos - 1, CHUNK)
                lhsT = coef_tiles[ci][:, cc * E:(cc + 1) * E]
            for bt in range(nbt):
                nc.tensor.matmul(
                    psums[bt][:, :],
                    lhsT,
                    rhs_tile[:, col_base + bt * BT:col_base + bt * BT + BT],
                    start=False,
                    stop=(pos == NPACK - 1),
                )

        # pack 0: x^2 on the scalar engine
        PK0 = p0_pool.tile([D, BLOC], b16)
        nc.scalar.square(PK0[:, :], ROTS[:, 0:BLOC])
        emit_matmuls(0, PK0, 0)

        # mixed-radix product groups, all on the vector engine
        pos = 1
        for gi, (lo, hi, s1) in enumerate(GROUPS):
            w = hi - lo
            PKg = g_pool.tile([D, w * BLOC], b16, name=f"PKg{gi}", tag="g")
            nc.vector.tensor_mul(
                PKg[:, :].rearrange("p (s b) -> p s b", s=w),
                R3[:, lo:hi, :],
                R3[:, s1:s1 + 1, :].broadcast_to((D, w, BLOC)),
            )
            for i in range(w):
                emit_matmuls(pos, PKg, i * BLOC)
                pos += 1

        # evict both PSUM chains into one SBUF tile (vector + scalar engine
        # in parallel), then a single output DMA trigger
        OT = out_pool.tile([E, BLOC], f32)
        nc.vector.tensor_scalar_add(OT[:, 0:BT], psums[0][:, :], TV[:, 0:1])
        nc.scalar.activation(OT[:, BT:BLOC], psums[1][:, :], Ident,
                             bias=TV[:, 0:1])
        nc.sync.dma_start(out_d.ap()[:, :], OT[:, :])

    nc.compile()
    return nc


def _host_precompute(Centroids: np.ndarray, Sigmas: np.ndarray):
    """Coefficient packs from the (replicated) small parameters."""
    Sinv = np.linalg.inv(Sigmas.astype(np.float64))
    A = 0.5 * (Sinv + np.swapaxes(Sinv, 1, 2))          # [E, D, D] symmetric
    c = Centroids[:, 0, :].astype(np.float64)           # [E, D]
    Ac = np.einsum("edk,ek->ed", A, c)

    cw = np.zeros((D, NPACK, E), np.float32)            # [row, emission pos, e]
    idx = np.arange(D)
    for posi, j in enumerate(ORDER):
        s = 2.0 if 1 <= j <= 63 else 1.0
        a = AVAL[posi]
        cw[:, posi, :] = s * A[:, (idx + a) % D, (idx + a + j) % D].T
    cw0_host = np.ascontiguousarray(cw[:, 0, :]).astype(bf16)            # [D, E]
    cwf_host = np.ascontiguousarray(
        cw[:, 1:, :].reshape(D, (NPACK - 1) * E)
    ).astype(f8e4)                                                       # [D, 64*E]
    um_host = np.ascontiguousarray((-2.0 * Ac.T)).astype(bf16)           # [D, E]
    tv_host = np.ascontiguousarray(
        np.einsum("ed,ed->e", Ac, c).astype(np.float32)[:, None]
    )                                                                    # [E, 1]
    return cw0_host, cwf_host, um_host, tv_host


def _get_nc():
    if "nc" not in _STATE:
        os.environ.setdefault("JAX_COMPILATION_CACHE_DIR", "/root/.jax_cache")
        _STATE["nc"] = _build_module()
    return _STATE["nc"]


def _make_in_maps(x, Centroids, Sigmas):
    cw0_host, cwf_host, um_host, tv_host = _host_precompute(
        np.asarray(Centroids, np.float32), np.asarray(Sigmas, np.float32)
    )
    xT = np.ascontiguousarray(np.asarray(x, np.float32).T).astype(bf16)  # [D, B]
    in_maps = []
    for cidx in range(NCORES):
        xTs = np.ascontiguousarray(xT[:, cidx * BLOC:(cidx + 1) * BLOC])
        xrot = np.stack([np.roll(xTs, -r, axis=0) for r in ROTVALS])
        in_maps.append({
            "xT": xTs,
            "xrot": np.ascontiguousarray(xrot),
            "cw0": cw0_host,
            "cwf": cwf_host,
            "um": um_host,
            "tv": tv_host,
        })
    return in_maps


def _run_device(in_maps, trace=False):
    from concourse import bass_utils

    nc = _get_nc()
    return bass_utils.run_bass_kernel_spmd(
        nc, in_maps, core_ids=list(range(NCORES)), trace=trace
    )


def kernel(x, Centroids, Sigmas):
    in_maps = _make_in_maps(x, Centroids, Sigmas)
    res = _run_device(in_maps)
    outT = np.concatenate([res.results[c]["out"] for c in range(NCORES)], axis=1)
    return np.ascontiguousarray(outT.T).astype(np.float32)
